# revision 23
# baseline (speedup 1.0000x reference)
"""DepthSelfAttention Trainium2 kernel v2 (8-core data-parallel SPMD).

Pos-major design: Q/K/V GEMMs run with kvt chunks as the stationary operand so
outputs land as [positions, features]. RoPE is folded into the K weights
(score = q_hat^T (R_q^T R_dep) k_hat), rmsnorm/scores/softmax are free-axis
DVE ops, attention-weighted V accumulation is fused scalar_tensor_tensor.

Contract: kernel(**inputs) takes FULL unsharded numpy inputs, returns the
FULL [4, 2048, 2048] fp32 output. Hardcoded for the problem-spec shapes.
"""

import sys

sys.path.insert(0, "/opt/trn_rl_repo")

from contextlib import ExitStack, nullcontext

import numpy as np

import concourse.bass as bass
import concourse.tile as tile
from concourse import bacc, mybir

F16 = mybir.dt.float16
F32 = mybir.dt.float32

DIM = 2048
NH = 16
NKV = 4
HD = 128
DEP = 8  # 7 history + current
NCORES = 8
EPS = 1.1920929e-07
SCALE = 1.0 / float(np.sqrt(HD))
ROPE_BASE = 10000.0
MAX_DEPTH = 16
DC = DIM // 128  # 16 contraction chunks
EXP_SHIFT = -4.0  # exp(s + EXP_SHIFT): softmax-invariant, tames fp16 range

_PROGRAM_CACHE = {}


# ---------------------------------------------------------------- host tables
def _rope_tables():
    inv_freq = 1.0 / ROPE_BASE ** (
        np.arange(0, HD, 2, dtype=np.float64) / HD
    )  # [64]
    pos = np.arange(MAX_DEPTH, dtype=np.float64)
    rpos = np.arange(MAX_DEPTH - 1, -1, -1, dtype=np.float64)
    fw = np.outer(pos, inv_freq)
    rv = np.outer(rpos, inv_freq)
    return np.cos(fw), np.sin(fw), np.cos(rv), np.sin(rv)  # each [16, 64]


def _rot_matrix(dcos, dsin, rcos, rsin):
    """R s.t. rope(v) = R @ v for one 128-dim head, given table rows [64]."""
    h = HD // 2
    R = np.zeros((HD, HD), np.float64)
    for m in range(h):
        R[m, m] = dcos[m]
        R[m, m + h] = dsin[m]
    for j in range(h):
        R[h + j, j] = -rsin[j]
        R[h + j, h + j] = rcos[j]
    return R


def _host_constants(q_gain):
    ident = np.eye(128, dtype=np.float16)
    gain = np.asarray(q_gain, np.float64)  # [16]
    grow = np.broadcast_to(
        (gain * np.sqrt(HD)).astype(np.float32)[None, :], (128, NH)
    ).copy()  # logit = grow * u * rsqrt(q2+128eps) * rsqrt(k2+128eps)
    cols = np.zeros((128, 2), np.float32)
    cols[:, 0] = 128.0 * EPS
    cols[:, 1] = EXP_SHIFT
    return {"ident": ident, "grow": grow, "cols": cols}


def _prep_weights(Wq, Wk, Wv, Wproj):
    f16 = np.float16

    def t_part(w):  # [out, din] -> [128, din//128, out]
        wt = np.ascontiguousarray(np.asarray(w, np.float64).T)  # [din, out]
        no = wt.shape[1]
        return np.ascontiguousarray(
            wt.reshape(wt.shape[0] // 128, 128, no).transpose(1, 0, 2)
        ).astype(f16)

    DCOS, DSIN, RCOS, RSIN = _rope_tables()
    d = DEP - 1
    Rq = _rot_matrix(DCOS[d], DSIN[d], RCOS[d], RSIN[d])
    half = HD // 2
    # G_dep = Rq^T R_dep is 2x2-block over pairs (m, m+64); store the four
    # coefficient diagonals per dep: acoef[dep] = [B11, B12, B21, B22][64]
    acoef = np.empty((128, DEP, 4, half), np.float32)
    for dep in range(DEP):
        G = Rq.T @ _rot_matrix(DCOS[dep], DSIN[dep], RCOS[dep], RSIN[dep])
        m = np.arange(half)
        acoef[:, dep, 0, :] = G[m, m]
        acoef[:, dep, 1, :] = G[m, m + half]
        acoef[:, dep, 2, :] = G[m + half, m]
        acoef[:, dep, 3, :] = G[m + half, m + half]

    return {
        "wq": t_part(Wq),  # [128, 16, 2048]
        "wk": t_part(Wk),  # [128, 16, 512]
        "wv": t_part(Wv),  # [128, 16, 512]
        "wp": t_part(Wproj),  # [128, 16, 2048]
        "acoef": acoef.astype(np.float16),  # [128, 8, 4, 64]
    }


def _prep_kvt(x, depth_history):
    """-> kvt [8, 128, 16, N] fp16: kvt[dep, p, c, n] = kv[n, dep, 128*c+p]."""
    B, S, D = x.shape
    N = B * S
    xf = x.reshape(N, 1, D)
    dh = depth_history.reshape(N, DEP - 1, D)
    kv = np.concatenate([dh, xf], axis=1).astype(np.float16)  # [N, 8, D]
    kvt = kv.transpose(1, 2, 0).reshape(DEP, DC, 128, N).transpose(0, 2, 1, 3)
    return kvt  # strided view; sliced/contiguized per core


# ---------------------------------------------------------------- device code
def build_program(npc, reps=1):
    """One-core SPMD program: fused Q+K phase (block-major), then V+O."""
    blk = min(512, npc)  # kvt DMA tile width
    nblk = npc // blk
    spb = blk // 128  # subs per block
    nsub = npc // 128
    assert npc % 128 == 0 and npc % blk == 0

    nc = bacc.Bacc()
    kvt_d = nc.declare_dram_parameter("kvt", [DEP, 128, DC, npc], F16, isOutput=False)
    wq_d = nc.declare_dram_parameter("wq", [128, DC, DIM], F16, isOutput=False)
    wk_d = nc.declare_dram_parameter("wk", [128, DC, 512], F16, isOutput=False)
    ac_d = nc.declare_dram_parameter("acoef", [128, DEP, 4, HD // 2], F16, isOutput=False)
    wv_d = nc.declare_dram_parameter("wv", [128, DC, 512], F16, isOutput=False)
    wp_d = nc.declare_dram_parameter("wp", [128, DC, DIM], F16, isOutput=False)
    grow_d = nc.declare_dram_parameter("grow", [128, NH], F32, isOutput=False)
    id_d = nc.declare_dram_parameter("ident", [128, 128], F16, isOutput=False)
    cols_d = nc.declare_dram_parameter("cols", [128, 2], F32, isOutput=False)
    out_d = nc.declare_dram_parameter("out", [npc, DIM], F32, isOutput=True)

    AF = mybir.ActivationFunctionType
    ALU = mybir.AluOpType
    H2 = HD // 2
    KDEPS = [DEP - 1] + list(range(DEP - 1))  # dep 7 first (reuses Q's tile)

    with tile.TileContext(nc) as tc, ExitStack() as top:
        const = top.enter_context(tc.tile_pool(name="const", bufs=1))
        id_sb = const.tile([128, 128], F16, name="id_sb")
        nc.sync.dma_start(id_sb[:], id_d[:])
        grow_sb = const.tile([128, NH], F32, name="grow_sb")
        nc.sync.dma_start(grow_sb[:], grow_d[:])
        cols_sb = const.tile([128, 2], F32, name="cols_sb")
        nc.sync.dma_start(cols_sb[:], cols_d[:])
        ac_sb = const.tile([128, DEP, 4, H2], F16, name="ac_sb")
        nc.sync.dma_start(ac_sb[:], ac_d[:])
        eps_b = cols_sb[:, 0:1]
        shift_b = cols_sb[:, 1:2]

        rep_ctx = tc.For_i(0, reps, 1) if reps > 4 else nullcontext(None)
        with rep_ctx:
         for _rep in range(reps if reps <= 4 else 1):
          with tc.tile_pool(name="keep", bufs=1) as keep:
            cq = [keep.tile([128, NH], F32, name=f"cq{s}", tag=f"cq{s}")
                  for s in range(nsub)]
            sc = [keep.tile([128, NH, DEP], F32, name=f"sc{s}", tag=f"sc{s}")
                  for s in range(nsub)]
            k2 = [keep.tile([128, NKV, DEP], F32, name=f"k2{s}", tag=f"k2{s}")
                  for s in range(nsub)]
            attn = [keep.tile([128, NH, DEP], F32, name=f"at{s}", tag=f"at{s}")
                    for s in range(nsub)]

            with (
                tc.tile_pool(name="wkp", bufs=1) as wkp,
                tc.tile_pool(name="ksr", bufs=3) as ksr,
                tc.tile_pool(name="wqp", bufs=1) as wqp,
                tc.tile_pool(name="wvp", bufs=1) as wvp,
            ):
                wk_sb = wkp.tile([128, DC, 512], F16, name="wk_sb")
                wq_sb = wqp.tile([128, DC, DIM], F16, name="wq_sb")
                wv_sb = wvp.tile([128, DC, 512], F16, name="wv_sb")
                wp_sb = wq_sb  # wp overwrites wq after its last use

                # ---------------- Phase QK (fused, block-major) ----------
                qTp_ctx = tc.tile_pool(name="qTp", bufs=1)
                qTp = qTp_ctx.__enter__()
                qT = [qTp.tile([128, DIM], F16, name=f"qT{s}", tag=f"qT{s}")
                      for s in range(nsub)]
                with (
                    tc.tile_pool(name="qps", bufs=5, space="PSUM") as qps,
                    tc.tile_pool(name="kps", bufs=3, space="PSUM") as kps,
                    tc.tile_pool(name="qsb", bufs=1) as qsb,
                    tc.tile_pool(name="ksb", bufs=2) as ksb,
                    tc.tile_pool(name="prodp", bufs=1) as prodp,
                ):
                    t7_next = None
                    for b in range(nblk):
                        if t7_next is not None:
                            t7 = t7_next
                            t7_next = None
                        else:
                            t7 = ksr.tile([128, DC, blk], F16, name="t7", tag="kvt")
                            if b == 0:
                                # interleave t7/wq chunk DMAs so Q starts early
                                for cg in range(4):
                                    nc.sync.dma_start(
                                        t7[:, 4 * cg : 4 * cg + 4, :],
                                        kvt_d[DEP - 1, :, 4 * cg : 4 * cg + 4,
                                              b * blk : (b + 1) * blk],
                                    )
                                    for c in range(4 * cg, 4 * cg + 4):
                                        nc.sync.dma_start(
                                            wq_sb[:, c, :], wq_d[:, c, :]
                                        )
                            else:
                                nc.sync.dma_start(
                                    t7[:],
                                    kvt_d[DEP - 1, :, :, b * blk : (b + 1) * blk],
                                )
                        if b == 0:
                            nc.sync.dma_start(wk_sb[:], wk_d[:])
                            nc.sync.dma_start(wv_sb[:], wv_d[:])
                        # ---- Q GEMMs for this block ----
                        for si in range(spb):
                            s = b * spb + si
                            qp = [
                                qps.tile([128, 512], F32, name="q_ps", tag="qps")
                                for _ in range(4)
                            ]
                            for c in range(DC):
                                st = t7[:, c, si * 128 : (si + 1) * 128]
                                for j in range(4):
                                    nc.tensor.matmul(
                                        qp[j][:],
                                        st,
                                        wq_sb[:, c, j * 512 : (j + 1) * 512],
                                        start=(c == 0),
                                        stop=(c == DC - 1),
                                    )
                            for j in range(4):
                                nc.scalar.copy(
                                    qT[s][:, j * 512 : (j + 1) * 512], qp[j][:]
                                )
                            sq = qsb.tile([128, DIM], F16, name="sq", tag="sq")
                            nc.vector.tensor_mul(sq[:], qT[s][:], qT[s][:])
                            q2 = qsb.tile([128, NH], F32, name="q2", tag="q2")
                            nc.vector.tensor_reduce(
                                q2[:],
                                sq[:].rearrange("p (h d) -> p h d", h=NH),
                                axis=mybir.AxisListType.X,
                                op=ALU.add,
                            )
                            qn = qsb.tile([128, NH], F32, name="qn", tag="qn")
                            nc.scalar.activation(qn[:], q2[:], AF.Sqrt, bias=eps_b)
                            qr = qsb.tile([128, NH], F32, name="qr", tag="qr")
                            nc.vector.reciprocal(qr[:], qn[:])
                            nc.vector.tensor_mul(cq[s][:], qr[:], grow_sb[:])
                        # ---- K path: dep 7 first (reuse t7), then 0..6 ----
                        for di, dep in enumerate(KDEPS):
                            if dep == DEP - 1:
                                kvt = t7
                            else:
                                kvt = ksr.tile(
                                    [128, DC, blk], F16, name="kvt_t", tag="kvt"
                                )
                                nc.sync.dma_start(
                                    kvt[:],
                                    kvt_d[dep, :, :, b * blk : (b + 1) * blk],
                                )
                                if b == nblk - 1:
                                    cc = 2 * (di - 1)
                                    nc.sync.dma_start(
                                        wp_sb[:, cc : cc + 2, :],
                                        wp_d[:, cc : cc + 2, :],
                                    )
                                    if di == DEP - 1:
                                        nc.sync.dma_start(
                                            wp_sb[:, 14:16, :], wp_d[:, 14:16, :]
                                        )
                                if di == 5 and b + 1 < nblk:
                                    t7_next = ksr.tile(
                                        [128, DC, blk], F16, name="t7",
                                        tag="kvt",
                                    )
                                    nc.sync.dma_start(
                                        t7_next[:],
                                        kvt_d[DEP - 1, :, :,
                                              (b + 1) * blk : (b + 2) * blk],
                                    )
                            a11 = ac_sb[:, dep, 0, :].unsqueeze(1).broadcast_to(
                                [128, NKV, H2])
                            a12 = ac_sb[:, dep, 1, :].unsqueeze(1).broadcast_to(
                                [128, NKV, H2])
                            a21 = ac_sb[:, dep, 2, :].unsqueeze(1).broadcast_to(
                                [128, NKV, H2])
                            a22 = ac_sb[:, dep, 3, :].unsqueeze(1).broadcast_to(
                                [128, NKV, H2])
                            for si in range(spb):
                                s = b * spb + si
                                kp = kps.tile([128, 512], F32, name="k_ps", tag="kps")
                                for c in range(DC):
                                    nc.tensor.matmul(
                                        kp[:],
                                        kvt[:, c, si * 128 : (si + 1) * 128],
                                        wk_sb[:, c, :],
                                        start=(c == 0),
                                        stop=(c == DC - 1),
                                    )
                                kf = ksb.tile([128, NKV, 2, H2], F16, name="kf",
                                              tag="kf")
                                nc.scalar.copy(
                                    kf[:].rearrange("p g t d -> p (g t d)"), kp[:]
                                )
                                ks2 = prodp.tile([128, NKV, 2, H2], F16, name="ks2",
                                               tag="ks2")
                                for g in range(NKV):
                                    nc.scalar.activation(
                                        ks2[:, g],
                                        kf[:, g],
                                        AF.Square,
                                        accum_out=k2[s][:, g, dep : dep + 1],
                                    )
                                kt = ksb.tile([128, NKV, 2, H2], F16, name="kt",
                                              tag="kt")
                                ktm = ksb.tile([128, NKV, H2], F16, name="ktm",
                                               tag="ktm")
                                nc.gpsimd.tensor_mul(
                                    kt[:, :, 0, :], kf[:, :, 0, :], a11)
                                nc.gpsimd.tensor_mul(ktm[:], kf[:, :, 1, :], a12)
                                nc.gpsimd.tensor_add(
                                    kt[:, :, 0, :], kt[:, :, 0, :], ktm[:])
                                ktm2 = ksb.tile([128, NKV, H2], F16, name="ktm2",
                                                tag="ktm2")
                                nc.gpsimd.tensor_mul(
                                    kt[:, :, 1, :], kf[:, :, 0, :], a21)
                                nc.gpsimd.tensor_mul(ktm2[:], kf[:, :, 1, :], a22)
                                nc.gpsimd.tensor_add(
                                    kt[:, :, 1, :], kt[:, :, 1, :], ktm2[:])
                                prod = prodp.tile([128, DIM], F16, name="prod",
                                                  tag="pr")
                                ktb = (
                                    kt[:]
                                    .rearrange("p g t d -> p g (t d)")
                                    .unsqueeze(2)
                                    .broadcast_to([128, NKV, 4, HD])
                                )
                                nc.vector.tensor_mul(
                                    prod[:].rearrange(
                                        "p (g hg d) -> p g hg d", g=NKV, hg=4
                                    ),
                                    qT[s][:].rearrange(
                                        "p (g hg d) -> p g hg d", g=NKV, hg=4
                                    ),
                                    ktb,
                                )
                                nc.vector.tensor_reduce(
                                    sc[s][:, :, dep : dep + 1].rearrange(
                                        "p h one -> p (h one)"
                                    ),
                                    prod[:].rearrange("p (h d) -> p h d", h=NH),
                                    axis=mybir.AxisListType.X,
                                    op=ALU.add,
                                )
                        # ---- softmax for this block's subs ----
                        for si in range(spb):
                            s = b * spb + si
                            kn = prodp.tile([128, NKV * DEP], F32, name="kn", tag="kn")
                            nc.scalar.activation(
                                kn[:],
                                k2[s][:].rearrange("p g d -> p (g d)"),
                                AF.Sqrt,
                                bias=eps_b,
                            )
                            kr = prodp.tile([128, NKV * DEP], F32, name="kr", tag="kr")
                            nc.vector.reciprocal(kr[:], kn[:])
                            t1 = prodp.tile([128, NH, DEP], F32, name="t1", tag="t1")
                            nc.vector.tensor_mul(
                                t1[:],
                                sc[s][:],
                                cq[s][:].unsqueeze(2).broadcast_to([128, NH, DEP]),
                            )
                            t2 = prodp.tile([128, NKV, 4, DEP], F32, name="t2",
                                          tag="t2")
                            nc.vector.tensor_mul(
                                t2[:],
                                t1[:].rearrange("p (g hg) d -> p g hg d", g=NKV),
                                kr[:]
                                .rearrange("p (g d) -> p g d", g=NKV)
                                .unsqueeze(2)
                                .broadcast_to([128, NKV, 4, DEP]),
                            )
                            epx = prodp.tile([128, NH, DEP], F32, name="epx",
                                           tag="epx")
                            nc.scalar.activation(
                                epx[:],
                                t2[:].rearrange("p g hg d -> p (g hg) d"),
                                AF.Exp,
                                bias=shift_b,
                            )
                            dsum = prodp.tile([128, NH], F32, name="dsum", tag="dsum")
                            nc.vector.tensor_reduce(
                                dsum[:], epx[:], axis=mybir.AxisListType.X,
                                op=ALU.add,
                            )
                            rds = prodp.tile([128, NH], F32, name="rds", tag="rds")
                            nc.vector.reciprocal(rds[:], dsum[:])
                            nc.vector.tensor_mul(
                                attn[s][:],
                                epx[:],
                                rds[:].unsqueeze(2).broadcast_to([128, NH, DEP]),
                            )
                qTp_ctx.__exit__(None, None, None)

                # ---------------- Phase V + O ----------------
                with (
                    tc.tile_pool(name="vps", bufs=2, space="PSUM") as vps,
                    tc.tile_pool(name="vsb", bufs=2) as vsb,
                    tc.tile_pool(name="yp", bufs=1) as yp,
                    tc.tile_pool(name="ytp", bufs=1, space="PSUM") as ytp,
                    tc.tile_pool(name="ysp", bufs=2) as ysp,
                    tc.tile_pool(name="ops", bufs=1, space="PSUM") as opsp,
                    tc.tile_pool(name="osb", bufs=2) as osbp,
                ):
                    for b in range(nblk):
                        y_b = [
                            yp.tile([128, DIM], F16, name=f"y{si}", tag=f"y{si}")
                            for si in range(spb)
                        ]
                        for dep in range(DEP):
                            kvt2 = ksr.tile([128, DC, blk], F16, name="kvt2",
                                            tag="kvt")
                            if b == 0 and dep == 0:
                                for cg in range(4):
                                    nc.sync.dma_start(
                                        kvt2[:, 4 * cg : 4 * cg + 4, :],
                                        kvt_d[dep, :, 4 * cg : 4 * cg + 4,
                                              b * blk : (b + 1) * blk],
                                    )
                            else:
                                nc.sync.dma_start(
                                    kvt2[:],
                                    kvt_d[dep, :, :, b * blk : (b + 1) * blk],
                                )
                            for si in range(spb):
                                s = b * spb + si
                                vp = vps.tile([128, 512], F32, name="v_ps",
                                              tag="vps")
                                for c in range(DC):
                                    nc.tensor.matmul(
                                        vp[:],
                                        kvt2[:, c, si * 128 : (si + 1) * 128],
                                        wv_sb[:, c, :],
                                        start=(c == 0),
                                        stop=(c == DC - 1),
                                    )
                                v_sb = vsb.tile([128, 512], F16, name="v_sb",
                                                tag="v")
                                nc.scalar.copy(v_sb[:], vp[:])
                                y = y_b[si]
                                ytmp = vsb.tile([128, 128], F16, name="ytmp",
                                                tag="ytmp")
                                # heads 0-11: per-group DVE tensor ops
                                # (2-free-dim broadcasts, HW-proven family)
                                for g in range(3):
                                    yg = y[:, g * 512 : (g + 1) * 512
                                           ].rearrange("p (hg d) -> p hg d",
                                                       hg=4)
                                    vg = (v_sb[:, g * 128 : (g + 1) * 128]
                                          .unsqueeze(1)
                                          .broadcast_to([128, 4, HD]))
                                    ag = (attn[s][:, 4 * g : 4 * g + 4, dep]
                                          .unsqueeze(2)
                                          .broadcast_to([128, 4, HD]))
                                    if dep == 0:
                                        nc.vector.tensor_mul(yg, vg, ag)
                                    else:
                                        ygt = vsb.tile(
                                            [128, 4, HD], F16,
                                            name=f"ygt{g}", tag=f"ygt{g}",
                                        )
                                        nc.vector.tensor_mul(ygt[:], vg, ag)
                                        nc.vector.tensor_add(yg, yg, ygt[:])
                                # heads 12-13 DVE STT, 14-15 Pool (baseline
                                # per-head op shapes)
                                for h in range(12, NH):
                                    vsl = v_sb[:, 3 * 128 : 4 * 128]
                                    ysl = y[:, h * 128 : (h + 1) * 128]
                                    a_sl = attn[s][:, h, dep : dep + 1]
                                    if h < 14:
                                        if dep == 0:
                                            nc.vector.tensor_scalar_mul(
                                                ysl, vsl, a_sl)
                                        else:
                                            nc.vector.scalar_tensor_tensor(
                                                ysl, vsl, a_sl, ysl,
                                                op0=ALU.mult, op1=ALU.add,
                                            )
                                    else:
                                        a_bc = a_sl.broadcast_to([128, 128])
                                        if dep == 0:
                                            nc.gpsimd.tensor_mul(ysl, vsl, a_bc)
                                        else:
                                            nc.gpsimd.tensor_mul(
                                                ytmp[:], vsl, a_bc)
                                            nc.gpsimd.tensor_add(
                                                ysl, ysl, ytmp[:])
                        # O projection per sub of this block
                        for si in range(spb):
                            s = b * spb + si
                            y = y_b[si]
                            ytps = ytp.tile([128, DC, 128], F16, name="yt_ps",
                                            tag="ytp")
                            for fc in range(DC):
                                nc.tensor.transpose(
                                    ytps[:, fc, :],
                                    y[:, fc * 128 : (fc + 1) * 128],
                                    id_sb[:],
                                )
                            ysb = ysp.tile([128, DC, 128], F16, name="ysb",
                                           tag="ysb")
                            for q4 in range(4):
                                if q4 != 3:
                                    nc.scalar.copy(
                                        ysb[:, 4 * q4 : 4 * q4 + 4, :],
                                        ytps[:, 4 * q4 : 4 * q4 + 4, :],
                                    )
                                else:
                                    nc.vector.tensor_copy(
                                        ysb[:, 4 * q4 : 4 * q4 + 4, :],
                                        ytps[:, 4 * q4 : 4 * q4 + 4, :],
                                    )
                            row = s * 128
                            opt = [
                                opsp.tile(
                                    [128, 512], F32, name=f"o_ps{j}",
                                    tag=f"og{j}",
                                )
                                for j in range(4)
                            ]
                            # fc outer so each transposed-y stationary is
                            # reused by all 4 output column groups
                            for fc in range(DC):
                                for j in range(4):
                                    nc.tensor.matmul(
                                        opt[j][:],
                                        ysb[:, fc, :],
                                        wp_sb[:, fc, j * 512 : (j + 1) * 512],
                                        start=(fc == 0),
                                        stop=(fc == DC - 1),
                                    )
                            for j in range(4):
                                ost = osbp.tile([128, 512], F32, name="ost",
                                                tag="ost")
                                if j % 2 == 0:
                                    nc.scalar.copy(ost[:], opt[j][:])
                                else:
                                    nc.vector.tensor_copy(ost[:], opt[j][:])
                                nc.sync.dma_start(
                                    out_d[row : row + 128, j * 512 : (j + 1) * 512],
                                    ost[:],
                                )
    nc.finalize()
    return nc


# ---------------------------------------------------------------- pjrt runner
class _Runner:
    """Persistent jitted shard_map executor (no output donation, so the
    compiled callable can be re-invoked for timing)."""

    def __init__(self, nc, n_cores):
        import jax
        from jax.experimental.shard_map import shard_map
        from jax.sharding import Mesh, NamedSharding, PartitionSpec

        from concourse import bass2jax

        bass2jax.install_neuronx_cc_hook()
        self.jax = jax
        self.nc = nc
        self.n_cores = n_cores

        in_names, out_names, out_avals = [], [], []
        partition_name = (
            nc.partition_id_tensor.name if nc.partition_id_tensor else None
        )
        for alloc in nc.m.functions[0].allocations:
            if not isinstance(alloc, mybir.MemoryLocationSet):
                continue
            name = alloc.memorylocations[0].name
            if alloc.kind == "ExternalInput":
                if name != partition_name:
                    in_names.append(name)
            elif alloc.kind == "ExternalOutput":
                out_names.append(name)
                shape = tuple(alloc.tensor_shape)
                dtype = mybir.dt.np(alloc.dtype)
                out_avals.append(jax.core.ShapedArray(shape, dtype))
        self.param_names = list(in_names)
        self.out_names = list(out_names)
        self.out_avals = out_avals
        all_in_names = in_names + out_names
        if partition_name is not None:
            all_in_names.append(partition_name)

        def _body(*args):
            operands = list(args)
            if partition_name is not None:
                operands.append(bass2jax.partition_id_tensor())
            outs = bass2jax._bass_exec_p.bind(
                *operands,
                out_avals=tuple(out_avals),
                in_names=tuple(all_in_names),
                out_names=tuple(out_names),
                lowering_input_output_aliases=(),
                sim_require_finite=True,
                sim_require_nnan=True,
                nc=nc,
            )
            return tuple(outs)

        devices = jax.devices()[:n_cores]
        assert len(devices) == n_cores
        self.mesh = Mesh(np.asarray(devices), ("core",))
        spec = PartitionSpec("core")
        n_all = len(self.param_names) + len(out_names)
        self.sharding = NamedSharding(self.mesh, spec)
        self.fn = jax.jit(
            shard_map(
                _body,
                mesh=self.mesh,
                in_specs=(spec,) * n_all,
                out_specs=(spec,) * len(out_names),
                check_rep=False,
            ),
            keep_unused=True,
        )
        self.dev_args = None

    def put(self, in_maps):
        jax = self.jax
        concat = [
            np.concatenate([np.asarray(m[name]) for m in in_maps], axis=0)
            for name in self.param_names
        ]
        zeros = [
            np.zeros((self.n_cores * a.shape[0], *a.shape[1:]), a.dtype)
            for a in self.out_avals
        ]
        self.dev_args = [
            jax.device_put(a, self.sharding) for a in (concat + zeros)
        ]
        jax.block_until_ready(self.dev_args)

    def run(self):
        outs = self.fn(*self.dev_args)
        self.jax.block_until_ready(outs)
        return outs

    def time_exec(self, iters=20):
        import time as _t

        self.run()  # warm
        times = []
        for _ in range(iters):
            t0 = _t.perf_counter()
            self.run()
            times.append(_t.perf_counter() - t0)
        return times


_RUNNER = None


# ---------------------------------------------------------------- entry point
def kernel(x, depth_history, Wq, Wk, Wv, Wproj, q_gain):
    global _RUNNER
    x = np.asarray(x, np.float32)
    depth_history = np.asarray(depth_history, np.float32)
    B, S, D = x.shape
    N = B * S
    npc = N // NCORES

    consts = _host_constants(np.asarray(q_gain, np.float32))
    weights = _prep_weights(
        np.asarray(Wq, np.float32),
        np.asarray(Wk, np.float32),
        np.asarray(Wv, np.float32),
        np.asarray(Wproj, np.float32),
    )
    kvt = _prep_kvt(x, depth_history)  # [8, 128, 16, N] strided fp16

    key = npc
    if key not in _PROGRAM_CACHE:
        _PROGRAM_CACHE[key] = build_program(npc)
    nc = _PROGRAM_CACHE[key]

    shared = dict(weights)
    shared.update(consts)
    in_maps = []
    for core in range(NCORES):
        m = dict(shared)
        m["kvt"] = np.ascontiguousarray(
            kvt[:, :, :, core * npc : (core + 1) * npc]
        )
        in_maps.append(m)

    if _RUNNER is None or _RUNNER.nc is not nc:
        _RUNNER = _Runner(nc, NCORES)
    _RUNNER.put(in_maps)
    try:
        outs = _RUNNER.run()
    except Exception:
        # wedged device / transient axon failure: rebuild runner, retry once
        _RUNNER = _Runner(nc, NCORES)
        _RUNNER.put(in_maps)
        outs = _RUNNER.run()
    oidx = _RUNNER.out_names.index("out")
    out = np.asarray(outs[oidx])
    return out.reshape(B, S, D).astype(np.float32)

